# revision 9
# baseline (speedup 1.0000x reference)
"""Trainium2 Bass kernel for a GQA attention layer (S=2048, HID=4096, H=32, KV=8, D=128).

Sharding: tensor-parallel over heads across 8 NeuronCores. Core c computes
q heads [4c, 4c+4) and kv head c end-to-end (QKV proj -> RMSNorm -> RoPE ->
causal flash-style attention -> partial o_proj). Each core returns a partial
[S, HID] o_proj output (w_o column-sharded); the host sums the 8 partials.

Device layout notes:
- All projections run with the feature dim on PSUM partitions: qkv_out[f, s] =
  (w_qkvT tile).T @ hidden_T tile, so q/k arrive as [D, S] (head dim on
  partitions), which is exactly the layout the scores matmul needs
  (contraction over D).
- Scores are computed transposed: scoresT[k, q] via lhsT=kT tile [D, Sk-128],
  rhs=qT [D, Sq-512]. The softmax denominator is ones[128,128].T @ exp(scoresT),
  which also broadcasts the per-q sum across all 128 partitions so the
  normalization multiply needs no cross-partition traffic. No max-subtraction:
  scores are O(5) for RMS-normed q/k, so exp is safe in fp32.
- Causal masking: blocks above the diagonal are skipped outright;
  multiplicative 0/1 masks are applied to the 4 diagonal-band tiles per
  (head, sq-chunk) after exp.
- Matmuls run in float32r (full PE rate at free-dim >= 256, ~tf32 precision).
"""

import numpy as np

import concourse.bass as bass
from concourse import bacc
import concourse.tile as tile
from concourse import mybir
from concourse.bass_utils import run_bass_kernel_spmd
from concourse.masks import make_identity

F32 = mybir.dt.float32
F32R = mybir.dt.float32r

S = 2048
HID = 4096
H = 32
KV = 8
D = 128
QH = H // KV  # 4 q heads per kv head; with 8 cores -> 1 kv head per core
NCORES = 8
EPS = 1e-6
THETA = 10000.0
HALF = D // 2

ST = 512  # seq chunk (matmul free dim)
N_ST = S // ST  # 4
KT = HID // D  # 32 contraction tiles for qkv
NB = QH + 2  # 6 feature blocks per core: q0..q3, k, v
SK = S // D  # 16 key blocks of 128
SCALE = float(D) ** -0.5


def build_bass():
    nc = bacc.Bacc(
        "TRN2", target_bir_lowering=False, debug=False, num_devices=NCORES
    )
    hid_d = nc.dram_tensor("hidden_t", [HID, S], F32R, kind="ExternalInput").ap()
    wqkv_d = nc.dram_tensor("w_qkvT", [HID, NB * D], F32R, kind="ExternalInput").ap()
    wo_d = nc.dram_tensor("w_oT", [QH * D, HID], F32R, kind="ExternalInput").ap()
    cosq_d = nc.dram_tensor("cosq", [D, S], F32, kind="ExternalInput").ap()
    sinq_d = nc.dram_tensor("sinq", [D, S], F32, kind="ExternalInput").ap()
    cosk_d = nc.dram_tensor("cosk", [D, S], F32, kind="ExternalInput").ap()
    sink_d = nc.dram_tensor("sink", [D, S], F32, kind="ExternalInput").ap()
    mask_d = nc.dram_tensor("masks", [4, D, ST], F32R, kind="ExternalInput").ap()
    out_d = nc.dram_tensor("out_partial", [S, HID], F32, kind="ExternalOutput").ap()

    with tile.TileContext(nc) as tc:
        build_kernel(
            nc, tc, hid_d, wqkv_d, wo_d,
            (cosq_d, sinq_d, cosk_d, sink_d), mask_d, out_d,
        )
    nc.finalize()
    return nc


def build_kernel(nc, tc, hid_d, wqkv_d, wo_d, tables_d, mask_d, out_d):
    from contextlib import ExitStack

    cosq_d, sinq_d, cosk_d, sink_d = tables_d

    with ExitStack() as outer:
        # ---- persistent: q/k in [D, S] f32r, v in [Sk, D] f32r, constants ----
        qk_pool = outer.enter_context(tc.tile_pool(name="qk", bufs=1))
        const_pool = outer.enter_context(tc.tile_pool(name="const", bufs=1))

        qkT = [qk_pool.tile([D, S], F32R, name=f"qkT_{m}") for m in range(QH + 1)]
        v_sb = qk_pool.tile([D, SK, D], F32R, name="v_sb")

        identity = const_pool.tile([D, D], F32, name="identity")
        make_identity(nc, identity[:])
        ones_f32 = const_pool.tile([D, D], F32, name="ones_f32")
        nc.vector.memset(ones_f32[:], 1.0)
        ones_sb = const_pool.tile([D, D], F32R, name="ones_sb")
        nc.vector.tensor_copy(ones_sb[:], ones_f32[:])
        eps_sb = const_pool.tile([D, 1], F32, name="eps_sb")
        nc.vector.memset(eps_sb[:], EPS)

        # ================= Phase 1: QKV + RMSNorm + RoPE =================
        with ExitStack() as p1:
            w_pool = p1.enter_context(tc.tile_pool(name="wqkv", bufs=1))
            hid_pool = p1.enter_context(tc.tile_pool(name="hid", bufs=4))
            tbl_pool = p1.enter_context(tc.tile_pool(name="tbl", bufs=1))
            tmp_pool = p1.enter_context(tc.tile_pool(name="p1tmp", bufs=2))
            ps_pool = p1.enter_context(tc.tile_pool(name="p1ps", bufs=1, space="PSUM"))

            wqkv_sb = w_pool.tile([D, KT, NB * D], F32R, name="wqkv_sb")
            nc.sync.dma_start(
                wqkv_sb[:], wqkv_d.rearrange("(kt p) m -> p kt m", p=D)
            )

            for st in range(N_ST):
                ssl = bass.ts(st, ST)
                # load hidden tiles for this seq chunk
                hid_t = []
                for kt in range(KT):
                    t = hid_pool.tile([D, ST], F32R, name="hid_t")
                    nc.sync.dma_start(t[:], hid_d[bass.ts(kt, D), ssl])
                    hid_t.append(t)

                # 6 accumulating matmul groups (one psum bank each)
                qkv_ps = [
                    ps_pool.tile([D, ST], F32, name=f"qkv_ps_{m}") for m in range(NB)
                ]
                for kt in range(KT):
                    for m in range(NB):
                        nc.tensor.matmul(
                            qkv_ps[m][:],
                            wqkv_sb[:, kt, bass.ts(m, D)],
                            hid_t[kt][:],
                            start=(kt == 0),
                            stop=(kt == KT - 1),
                        )

                # rope table slices for this chunk
                tbls = []
                for i, td in enumerate((cosq_d, sinq_d, cosk_d, sink_d)):
                    t = tbl_pool.tile([D, ST], F32, name=f"tbl_{i}")
                    nc.sync.dma_start(t[:], td[:, ssl])
                    tbls.append(t)
                cosq_t, sinq_t, cosk_t, sink_t = tbls

                # q/k blocks: rmsnorm + rope
                for m in range(QH + 1):
                    cos_t, sin_t = (cosq_t, sinq_t) if m < QH else (cosk_t, sink_t)
                    ps = qkv_ps[m]
                    raw = tmp_pool.tile([D, ST], F32, name="raw")
                    nc.scalar.copy(raw[:], ps[:])
                    sq = tmp_pool.tile([D, ST], F32R, name="sq")
                    nc.vector.tensor_mul(sq[:], raw[:], raw[:])
                    msum = ps_pool.tile([D, ST], F32, name="msum_ps")
                    nc.tensor.matmul(
                        msum[:], ones_sb[:], sq[:], start=True, stop=True
                    )
                    # rstd = 1/sqrt(mean + eps), already broadcast on partitions
                    rstd = tmp_pool.tile([D, ST], F32, name="rstd")
                    nc.scalar.activation(
                        rstd[:], msum[:], mybir.ActivationFunctionType.Sqrt,
                        bias=eps_sb[:], scale=1.0 / D,
                    )
                    nc.vector.reciprocal(rstd[:], rstd[:])
                    # rope: rot = raw*cos + swap(raw)*sin_signed (gains in tables)
                    t1 = tmp_pool.tile([D, ST], F32, name="t1")
                    nc.vector.tensor_mul(t1[:], raw[:], cos_t[:])
                    # sin table ships half-swapped so both DVE inputs share a
                    # partition base; only the output is relocated.
                    t2 = tmp_pool.tile([D, ST], F32, name="t2")
                    nc.vector.tensor_mul(t2[:HALF], raw[HALF:], sin_t[HALF:])
                    nc.vector.tensor_mul(t2[HALF:], raw[:HALF], sin_t[:HALF])
                    nc.vector.tensor_add(t1[:], t1[:], t2[:])
                    nc.vector.tensor_mul(qkT[m][:, ssl], t1[:], rstd[:])

                # v block: transpose [D, ST] -> 4x [128, 128] into v_sb
                vtmp = tmp_pool.tile([D, ST], F32, name="vtmp")
                nc.vector.tensor_copy(vtmp[:], qkv_ps[QH + 1][:])
                for c in range(ST // D):
                    tp = ps_pool.tile([D, D], F32, name="vt_ps")
                    nc.tensor.transpose(tp[:], vtmp[:, bass.ts(c, D)], identity[:])
                    nc.vector.tensor_copy(v_sb[:, st * (ST // D) + c, :], tp[:])

        # ============ Phases 2+3 share ctx and w_oT ============
        with ExitStack() as p23:
            ctx_pool = p23.enter_context(tc.tile_pool(name="ctx", bufs=1))
            wo_pool = p23.enter_context(tc.tile_pool(name="wo", bufs=1))
            ctx_sb = [ctx_pool.tile([D, S], F32R, name=f"ctx_{h}") for h in range(QH)]
            wo_sb = wo_pool.tile([D, QH, HID], F32R, name="wo_sb")
            nc.sync.dma_start(wo_sb[:], wo_d.rearrange("(h p) n -> p h n", p=D))

            # ---------------- Phase 2: attention ----------------
            with ExitStack() as p2:
                m_pool = p2.enter_context(tc.tile_pool(name="maskp", bufs=1))
                e_pool = p2.enter_context(tc.tile_pool(name="expp", bufs=6))
                a_tmp = p2.enter_context(tc.tile_pool(name="p2tmp", bufs=3))
                sc_ps_pool = p2.enter_context(
                    tc.tile_pool(name="sc_ps", bufs=3, space="PSUM")
                )
                cd_ps_pool = p2.enter_context(
                    tc.tile_pool(name="cd_ps", bufs=2, space="PSUM")
                )

                mask_sb = m_pool.tile([D, 4, ST], F32R, name="mask_sb")
                nc.sync.dma_start(mask_sb[:], mask_d.rearrange("r p f -> p r f"))

                for h in range(QH):
                    for sq in range(N_ST):
                        ssl = bass.ts(sq, ST)
                        n_sk = (sq + 1) * (ST // D)
                        ctx_ps = cd_ps_pool.tile([D, ST], F32, name="ctx_ps")
                        den_ps = cd_ps_pool.tile([D, ST], F32, name="den_ps")
                        for ski in range(n_sk):
                            sc = sc_ps_pool.tile([D, ST], F32, name="sc_ps")
                            nc.tensor.matmul(
                                sc[:],
                                qkT[QH][:, bass.ts(ski, D)],
                                qkT[h][:, ssl],
                                start=True,
                                stop=True,
                            )
                            e_sb = e_pool.tile([D, ST], F32R, name="e_sb")
                            nc.scalar.activation(
                                e_sb[:], sc[:],
                                mybir.ActivationFunctionType.Exp,
                                scale=SCALE,
                            )
                            r = ski - sq * (ST // D)
                            if r >= 0:
                                nc.vector.tensor_mul(
                                    e_sb[:], e_sb[:], mask_sb[:, r, :]
                                )
                            first = ski == 0
                            last = ski == n_sk - 1
                            nc.tensor.matmul(
                                ctx_ps[:], v_sb[:, ski, :], e_sb[:],
                                start=first, stop=last,
                            )
                            nc.tensor.matmul(
                                den_ps[:], ones_sb[:], e_sb[:],
                                start=first, stop=last,
                            )
                        recip = a_tmp.tile([D, ST], F32, name="recip")
                        nc.vector.reciprocal(recip[:], den_ps[:])
                        nc.vector.tensor_mul(
                            ctx_sb[h][:, ssl], ctx_ps[:], recip[:]
                        )

            # ---------------- Phase 3: o_proj ----------------
            with ExitStack() as p3:
                o_pool = p3.enter_context(tc.tile_pool(name="outst", bufs=2))
                o_ps_pool = p3.enter_context(
                    tc.tile_pool(name="o_ps", bufs=4, space="PSUM")
                )
                for st in range(S // D):
                    out_sb = o_pool.tile([D, HID], F32, name="out_sb")
                    for nt in range(HID // ST):
                        ps = o_ps_pool.tile([D, ST], F32, name="o_ps")
                        for ht in range(QH):
                            nc.tensor.matmul(
                                ps[:],
                                ctx_sb[ht][:, bass.ts(st, D)],
                                wo_sb[:, ht, bass.ts(nt, ST)],
                                start=(ht == 0),
                                stop=(ht == QH - 1),
                            )
                        nc.vector.tensor_copy(out_sb[:, bass.ts(nt, ST)], ps[:])
                    nc.sync.dma_start(out_d[bass.ts(st, D), :], out_sb[:])


def _host_prep(positions, hidden_states, w_qkv, w_o, gq, gk):
    positions = np.asarray(positions)
    hidden_states = np.asarray(hidden_states, dtype=np.float32)
    w_qkv = np.asarray(w_qkv, dtype=np.float32)
    w_o = np.asarray(w_o, dtype=np.float32)
    gq = np.asarray(gq, dtype=np.float32)
    gk = np.asarray(gk, dtype=np.float32)

    hidden_t = np.ascontiguousarray(hidden_states.T)  # [HID, S]

    inv_freq = 1.0 / (THETA ** (np.arange(HALF, dtype=np.float32) * 2.0 / D))
    freqs = positions.astype(np.float32)[:, None] * inv_freq  # [S, HALF]
    cos = np.cos(freqs)
    sin = np.sin(freqs)

    def rope_tables(g):
        # cos_t[d, s] = cos[s, d % HALF] * g[d]
        cos_t = np.concatenate([cos * g[None, :HALF], cos * g[None, HALF:]], axis=1).T
        # rot[d] = x[d] cos[d] - x[d+HALF] sin[d]   (d < HALF)
        #          x[d] cos[d] + x[d-HALF] sin[d]   (d >= HALF)
        # The partner's gain is folded into the sin table, and the table ships
        # HALF-SWAPPED: row j holds the sin factor that multiplies x[j], i.e.
        # the factor for output row j^HALF. This keeps both DVE inputs of the
        # rope cross-multiply at the same partition base.
        sin_t = np.concatenate([sin * g[None, :HALF], -sin * g[None, HALF:]], axis=1).T
        return (
            np.ascontiguousarray(cos_t, dtype=np.float32),
            np.ascontiguousarray(sin_t, dtype=np.float32),
        )

    cosq, sinq = rope_tables(gq)
    cosk, sink = rope_tables(gk)

    # diagonal-band masks: mask[r][k, q] = 1 if (r*D + k) <= q else 0
    k_idx = np.arange(D)
    q_idx = np.arange(ST)
    masks = np.stack(
        [
            ((r * D + k_idx)[:, None] <= q_idx[None, :]).astype(np.float32)
            for r in range(4)
        ]
    )

    per_core = []
    for c in range(NCORES):
        q_rows = w_qkv[c * QH * D : (c + 1) * QH * D]  # [512, HID]
        k_rows = w_qkv[H * D + c * D : H * D + (c + 1) * D]  # [128, HID]
        v_rows = w_qkv[(H + KV) * D + c * D : (H + KV) * D + (c + 1) * D]
        wqkv_c = np.concatenate([q_rows, k_rows, v_rows], axis=0)  # [768, HID]
        wqkv_t = np.ascontiguousarray(wqkv_c.T)  # [HID, 768]
        wo_t = np.ascontiguousarray(w_o[:, c * QH * D : (c + 1) * QH * D].T)
        per_core.append(
            {
                "hidden_t": hidden_t,
                "w_qkvT": wqkv_t,
                "w_oT": wo_t,
                "cosq": cosq,
                "sinq": sinq,
                "cosk": cosk,
                "sink": sink,
                "masks": masks,
            }
        )
    return per_core


_NC_CACHE = {}


def _get_nc():
    if "nc" not in _NC_CACHE:
        _NC_CACHE["nc"] = build_bass()
    return _NC_CACHE["nc"]


def kernel(positions, hidden_states, w_qkv, w_o, gq, gk, _trace=False):
    in_maps = _host_prep(positions, hidden_states, w_qkv, w_o, gq, gk)
    nc = _get_nc()
    res = run_bass_kernel_spmd(
        nc, in_maps, core_ids=list(range(NCORES)), trace=_trace
    )
    out = np.zeros((S, HID), dtype=np.float32)
    for r in res.results:
        out += r["out_partial"]
    if _trace:
        kernel._last_results = res
    return out


# revision 14
# speedup vs baseline: 1.0389x; 1.0389x over previous
"""Trainium2 Bass kernel for a GQA attention layer (S=2048, HID=4096, H=32, KV=8, D=128).

Sharding: tensor-parallel over heads across 8 NeuronCores. Core c computes
q heads [4c, 4c+4) and kv head c end-to-end (QKV proj -> RMSNorm -> RoPE ->
causal flash-style attention -> partial o_proj). Each core returns a partial
[S, HID] o_proj output (w_o column-sharded); the host sums the 8 partials.

Device layout notes:
- All projections run with the feature dim on PSUM partitions: qkv_out[f, s] =
  (w_qkvT tile).T @ hidden_T tile, so q/k arrive as [D, S] (head dim on
  partitions), which is exactly the layout the scores matmul needs
  (contraction over D).
- Scores are computed transposed: scoresT[k, q] via lhsT=kT tile [D, Sk-128],
  rhs=qT [D, Sq-512]. The softmax denominator is ones[128,128].T @ exp(scoresT),
  which also broadcasts the per-q sum across all 128 partitions so the
  normalization multiply needs no cross-partition traffic. No max-subtraction:
  scores are O(5) for RMS-normed q/k, so exp is safe in fp32.
- Causal masking: blocks above the diagonal are skipped outright;
  multiplicative 0/1 masks are applied to the 4 diagonal-band tiles per
  (head, sq-chunk) after exp.
- Matmul operands are bf16 (fp32 PSUM accumulation); the rmsnorm/rope/softmax
  normalization chain stays fp32. Set USE_BF16=False for float32r operands
  (~tf32 precision, 2x slower matmuls).
"""

import numpy as np

import concourse.bass as bass
from concourse import bacc
import concourse.tile as tile
from concourse import mybir
from concourse.bass_utils import run_bass_kernel_spmd
from concourse.masks import make_identity

F32 = mybir.dt.float32
F32R = mybir.dt.float32r
BF16 = mybir.dt.bfloat16

USE_BF16 = True
MM_DT = BF16 if USE_BF16 else F32R
# PE-transpose path dtype: f32r can't be memset/affine_select'd, use plain f32
TR_DT = BF16 if USE_BF16 else F32

S = 2048
HID = 4096
H = 32
KV = 8
D = 128
QH = H // KV  # 4 q heads per kv head; with 8 cores -> 1 kv head per core
NCORES = 8
EPS = 1e-6
THETA = 10000.0
HALF = D // 2

ST = 512  # seq chunk (matmul free dim)
N_ST = S // ST  # 4
KT = HID // D  # 32 contraction tiles for qkv
NB = QH + 2  # 6 feature blocks per core: q0..q3, k, v
SK = S // D  # 16 key blocks of 128
SCALE = float(D) ** -0.5


def build_bass():
    nc = bacc.Bacc(
        "TRN2", target_bir_lowering=False, debug=False, num_devices=NCORES
    )
    hid_d = nc.dram_tensor("hidden_t", [HID, S], MM_DT, kind="ExternalInput").ap()
    wqkv_d = nc.dram_tensor("w_qkvT", [HID, NB * D], MM_DT, kind="ExternalInput").ap()
    wo_d = nc.dram_tensor("w_oT", [QH * D, HID], MM_DT, kind="ExternalInput").ap()
    cosq_d = nc.dram_tensor("cosq", [D, S], F32, kind="ExternalInput").ap()
    sinq_d = nc.dram_tensor("sinq", [D, S], F32, kind="ExternalInput").ap()
    cosk_d = nc.dram_tensor("cosk", [D, S], F32, kind="ExternalInput").ap()
    sink_d = nc.dram_tensor("sink", [D, S], F32, kind="ExternalInput").ap()
    mask_d = nc.dram_tensor("masks", [4, D, ST], MM_DT, kind="ExternalInput").ap()
    out_d = nc.dram_tensor("out_partial", [S, HID], F32, kind="ExternalOutput").ap()

    with tile.TileContext(nc) as tc:
        build_kernel(
            nc, tc, hid_d, wqkv_d, wo_d,
            (cosq_d, sinq_d, cosk_d, sink_d), mask_d, out_d,
        )
    nc.finalize()
    return nc


def build_kernel(nc, tc, hid_d, wqkv_d, wo_d, tables_d, mask_d, out_d):
    from contextlib import ExitStack

    cosq_d, sinq_d, cosk_d, sink_d = tables_d

    with ExitStack() as outer:
        # ---- persistent: q/k in [D, S], v in [Sk, D], constants ----
        qk_pool = outer.enter_context(tc.tile_pool(name="qk", bufs=1))
        const_pool = outer.enter_context(tc.tile_pool(name="const", bufs=1))

        qkT = [qk_pool.tile([D, S], MM_DT, name=f"qkT_{m}") for m in range(QH + 1)]
        v_sb = qk_pool.tile([D, SK, D], MM_DT, name="v_sb")

        identity = const_pool.tile([D, D], TR_DT, name="identity")
        make_identity(nc, identity[:])
        ones_f32 = const_pool.tile([D, D], F32, name="ones_f32")
        nc.vector.memset(ones_f32[:], 1.0)
        ones_sb = const_pool.tile([D, D], MM_DT, name="ones_sb")
        nc.vector.tensor_copy(ones_sb[:], ones_f32[:])
        eps_sb = const_pool.tile([D, 1], F32, name="eps_sb")
        nc.vector.memset(eps_sb[:], EPS)

        # ================= Phase 1: QKV + RMSNorm + RoPE =================
        with ExitStack() as p1:
            w_pool = p1.enter_context(tc.tile_pool(name="wqkv", bufs=1))
            hid_pool = p1.enter_context(tc.tile_pool(name="hid", bufs=6))
            tbl_pool = p1.enter_context(tc.tile_pool(name="tbl", bufs=2))
            tmp_pool = p1.enter_context(tc.tile_pool(name="p1tmp", bufs=2))
            ps_pool = p1.enter_context(tc.tile_pool(name="p1ps", bufs=1, space="PSUM"))

            wqkv_sb = w_pool.tile([D, KT, NB * D], MM_DT, name="wqkv_sb")
            nc.sync.dma_start(
                wqkv_sb[:], wqkv_d.rearrange("(kt p) m -> p kt m", p=D)
            )

            for st in range(N_ST):
                ssl = bass.ts(st, ST)
                # load hidden tiles for this seq chunk
                hid_t = []
                for kt in range(KT):
                    t = hid_pool.tile([D, ST], MM_DT, name="hid_t")
                    nc.sync.dma_start(t[:], hid_d[bass.ts(kt, D), ssl])
                    hid_t.append(t)

                # 6 accumulating matmul groups (one psum bank each)
                qkv_ps = [
                    ps_pool.tile([D, ST], F32, name=f"qkv_ps_{m}") for m in range(NB)
                ]
                for kt in range(KT):
                    for m in range(NB):
                        nc.tensor.matmul(
                            qkv_ps[m][:],
                            wqkv_sb[:, kt, bass.ts(m, D)],
                            hid_t[kt][:],
                            start=(kt == 0),
                            stop=(kt == KT - 1),
                        )

                # rope table slices for this chunk
                tbls = []
                for i, td in enumerate((cosq_d, sinq_d, cosk_d, sink_d)):
                    t = tbl_pool.tile([D, ST], F32, name=f"tbl_{i}")
                    nc.sync.dma_start(t[:], td[:, ssl])
                    tbls.append(t)
                cosq_t, sinq_t, cosk_t, sink_t = tbls

                # q/k blocks: rmsnorm + rope
                for m in range(QH + 1):
                    cos_t, sin_t = (cosq_t, sinq_t) if m < QH else (cosk_t, sink_t)
                    ps = qkv_ps[m]
                    raw = tmp_pool.tile([D, ST], F32, name="raw")
                    nc.scalar.copy(raw[:], ps[:])
                    sq = tmp_pool.tile([D, ST], MM_DT, name="sq")
                    nc.vector.tensor_mul(sq[:], raw[:], raw[:])
                    msum = ps_pool.tile([D, ST], F32, name="msum_ps")
                    nc.tensor.matmul(
                        msum[:], ones_sb[:], sq[:], start=True, stop=True
                    )
                    # rstd = 1/sqrt(mean + eps), already broadcast on partitions
                    rstd = tmp_pool.tile([D, ST], F32, name="rstd")
                    nc.scalar.activation(
                        rstd[:], msum[:], mybir.ActivationFunctionType.Sqrt,
                        bias=eps_sb[:], scale=1.0 / D,
                    )
                    nc.vector.reciprocal_approx_fast(rstd[:], rstd[:])
                    # rope: rot = raw*cos + swap(raw)*sin_signed (gains in
                    # tables; sin table ships half-swapped so both DVE inputs
                    # share a partition base - only the output is relocated)
                    t1 = tmp_pool.tile([D, ST], F32, name="t1")
                    nc.vector.tensor_mul(t1[:], raw[:], cos_t[:])
                    t2 = tmp_pool.tile([D, ST], F32, name="t2")
                    nc.vector.tensor_mul(t2[:HALF], raw[HALF:], sin_t[HALF:])
                    nc.vector.tensor_mul(t2[HALF:], raw[:HALF], sin_t[:HALF])
                    nc.vector.tensor_add(t1[:], t1[:], t2[:])
                    nc.vector.tensor_mul(qkT[m][:, ssl], t1[:], rstd[:])

                # v block: transpose [D, ST] -> 4x [128, 128] into v_sb
                vtmp = tmp_pool.tile([D, ST], TR_DT, name="vtmp")
                nc.vector.tensor_copy(vtmp[:], qkv_ps[QH + 1][:])
                for c in range(ST // D):
                    tp = ps_pool.tile([D, D], TR_DT, name="vt_ps")
                    nc.tensor.transpose(tp[:], vtmp[:, bass.ts(c, D)], identity[:])
                    nc.vector.tensor_copy(v_sb[:, st * (ST // D) + c, :], tp[:])

        # ========== Phases 2+3 fused: attention + o_proj per chunk ==========
        with ExitStack() as p23:
            ctx_pool = p23.enter_context(tc.tile_pool(name="ctx", bufs=1))
            wo_pool = p23.enter_context(tc.tile_pool(name="wo", bufs=1))
            m_pool = p23.enter_context(tc.tile_pool(name="maskp", bufs=1))
            e_pool = p23.enter_context(tc.tile_pool(name="expp", bufs=6))
            a_tmp = p23.enter_context(tc.tile_pool(name="p2tmp", bufs=3))
            o_pool = p23.enter_context(tc.tile_pool(name="outst", bufs=4))
            sc_ps_pool = p23.enter_context(
                tc.tile_pool(name="sc_ps", bufs=2, space="PSUM")
            )
            cd_ps_pool = p23.enter_context(
                tc.tile_pool(name="cd_ps", bufs=2, space="PSUM")
            )
            o_ps_pool = p23.enter_context(
                tc.tile_pool(name="o_ps", bufs=2, space="PSUM")
            )

            ctx_sb = [ctx_pool.tile([D, S], MM_DT, name=f"ctx_{h}") for h in range(QH)]
            wo_sb = wo_pool.tile([D, QH, HID], MM_DT, name="wo_sb")
            nc.sync.dma_start(wo_sb[:], wo_d.rearrange("(h p) n -> p h n", p=D))
            mask_sb = m_pool.tile([D, 4, ST], MM_DT, name="mask_sb")
            nc.sync.dma_start(mask_sb[:], mask_d.rearrange("r p f -> p r f"))

            for sq in range(N_ST):
                ssl = bass.ts(sq, ST)
                n_sk = (sq + 1) * (ST // D)
                # ---- attention for all heads on this chunk ----
                for h in range(QH):
                    ctx_ps = cd_ps_pool.tile([D, ST], F32, name="ctx_ps")
                    den_ps = cd_ps_pool.tile([D, ST], F32, name="den_ps")
                    for ski in range(n_sk):
                        sc = sc_ps_pool.tile([D, ST], F32, name="sc_ps")
                        nc.tensor.matmul(
                            sc[:],
                            qkT[QH][:, bass.ts(ski, D)],
                            qkT[h][:, ssl],
                            start=True,
                            stop=True,
                        )
                        e_sb = e_pool.tile([D, ST], MM_DT, name="e_sb")
                        nc.scalar.activation(
                            e_sb[:], sc[:],
                            mybir.ActivationFunctionType.Exp,
                            scale=SCALE,
                        )
                        r = ski - sq * (ST // D)
                        if r >= 0:
                            nc.vector.tensor_mul(
                                e_sb[:], e_sb[:], mask_sb[:, r, :]
                            )
                        first = ski == 0
                        last = ski == n_sk - 1
                        nc.tensor.matmul(
                            ctx_ps[:], v_sb[:, ski, :], e_sb[:],
                            start=first, stop=last,
                        )
                        nc.tensor.matmul(
                            den_ps[:], ones_sb[:], e_sb[:],
                            start=first, stop=last,
                        )
                    recip = a_tmp.tile([D, ST], F32, name="recip")
                    nc.vector.reciprocal_approx_fast(recip[:], den_ps[:])
                    nc.vector.tensor_mul(ctx_sb[h][:, ssl], ctx_ps[:], recip[:])

                # ---- o_proj for the 4 row-tiles of this chunk ----
                for sti in range(ST // D):
                    st = sq * (ST // D) + sti
                    out_sb = o_pool.tile([D, HID], F32, name="out_sb")
                    for nt in range(HID // ST):
                        ps = o_ps_pool.tile([D, ST], F32, name="o_ps")
                        for ht in range(QH):
                            nc.tensor.matmul(
                                ps[:],
                                ctx_sb[ht][:, bass.ts(st, D)],
                                wo_sb[:, ht, bass.ts(nt, ST)],
                                start=(ht == 0),
                                stop=(ht == QH - 1),
                            )
                        nc.vector.tensor_copy(out_sb[:, bass.ts(nt, ST)], ps[:])
                    nc.sync.dma_start(out_d[bass.ts(st, D), :], out_sb[:])


def _host_prep(positions, hidden_states, w_qkv, w_o, gq, gk):
    import ml_dtypes

    mm_np = ml_dtypes.bfloat16 if USE_BF16 else np.float32

    positions = np.asarray(positions)
    hidden_states = np.asarray(hidden_states, dtype=np.float32)
    w_qkv = np.asarray(w_qkv, dtype=np.float32)
    w_o = np.asarray(w_o, dtype=np.float32)
    gq = np.asarray(gq, dtype=np.float32)
    gk = np.asarray(gk, dtype=np.float32)

    hidden_t = np.ascontiguousarray(hidden_states.T).astype(mm_np)  # [HID, S]

    inv_freq = 1.0 / (THETA ** (np.arange(HALF, dtype=np.float32) * 2.0 / D))
    freqs = positions.astype(np.float32)[:, None] * inv_freq  # [S, HALF]
    cos = np.cos(freqs)
    sin = np.sin(freqs)

    def rope_tables(g):
        # cos_t[d, s] = cos[s, d % HALF] * g[d]
        cos_t = np.concatenate([cos * g[None, :HALF], cos * g[None, HALF:]], axis=1).T
        # rot[d] = x[d] cos[d] - x[d+HALF] sin[d]   (d < HALF)
        #          x[d] cos[d] + x[d-HALF] sin[d]   (d >= HALF)
        # The partner's gain is folded into the sin table, and the table ships
        # HALF-SWAPPED: row j holds the sin factor that multiplies x[j], i.e.
        # the factor for output row j^HALF. This keeps both DVE inputs of the
        # rope cross-multiply at the same partition base.
        sin_t = np.concatenate([sin * g[None, :HALF], -sin * g[None, HALF:]], axis=1).T
        return (
            np.ascontiguousarray(cos_t, dtype=np.float32),
            np.ascontiguousarray(sin_t, dtype=np.float32),
        )

    cosq, sinq = rope_tables(gq)
    cosk, sink = rope_tables(gk)

    # diagonal-band masks: mask[r][k, q] = 1 if (r*D + k) <= q else 0
    k_idx = np.arange(D)
    q_idx = np.arange(ST)
    masks = np.stack(
        [
            ((r * D + k_idx)[:, None] <= q_idx[None, :]).astype(mm_np)
            for r in range(4)
        ]
    )

    per_core = []
    for c in range(NCORES):
        q_rows = w_qkv[c * QH * D : (c + 1) * QH * D]  # [512, HID]
        k_rows = w_qkv[H * D + c * D : H * D + (c + 1) * D]  # [128, HID]
        v_rows = w_qkv[(H + KV) * D + c * D : (H + KV) * D + (c + 1) * D]
        wqkv_c = np.concatenate([q_rows, k_rows, v_rows], axis=0)  # [768, HID]
        wqkv_t = np.ascontiguousarray(wqkv_c.T).astype(mm_np)  # [HID, 768]
        wo_t = np.ascontiguousarray(
            w_o[:, c * QH * D : (c + 1) * QH * D].T
        ).astype(mm_np)
        per_core.append(
            {
                "hidden_t": hidden_t,
                "w_qkvT": wqkv_t,
                "w_oT": wo_t,
                "cosq": cosq,
                "sinq": sinq,
                "cosk": cosk,
                "sink": sink,
                "masks": masks,
            }
        )
    return per_core


_NC_CACHE = {}


def _get_nc():
    if "nc" not in _NC_CACHE:
        _NC_CACHE["nc"] = build_bass()
    return _NC_CACHE["nc"]


def kernel(positions, hidden_states, w_qkv, w_o, gq, gk, _trace=False):
    in_maps = _host_prep(positions, hidden_states, w_qkv, w_o, gq, gk)
    nc = _get_nc()
    res = run_bass_kernel_spmd(
        nc, in_maps, core_ids=list(range(NCORES)), trace=_trace
    )
    out = np.zeros((S, HID), dtype=np.float32)
    for r in res.results:
        out += r["out_partial"]
    if _trace:
        kernel._last_results = res
    return out


# revision 17
# speedup vs baseline: 1.3433x; 1.2930x over previous
"""Trainium2 Bass kernel for a GQA attention layer (S=2048, HID=4096, H=32, KV=8, D=128).

Sharding: tensor-parallel over heads across 8 NeuronCores. Core c computes
q heads [4c, 4c+4) and kv head c end-to-end (QKV proj -> RMSNorm -> RoPE ->
causal flash-style attention -> partial o_proj). Each core returns a partial
[S, HID] o_proj output (w_o column-sharded); the host sums the 8 partials.

Device layout notes:
- All projections run with the feature dim on PSUM partitions: qkv_out[f, s] =
  (w_qkvT tile).T @ hidden_T tile, so q/k arrive as [D, S] (head dim on
  partitions), which is exactly the layout the scores matmul needs
  (contraction over D).
- Scores are computed transposed: scoresT[k, q] via lhsT=kT tile [D, Sk-128],
  rhs=qT [D, Sq-512]. The softmax denominator is ones[128,128].T @ exp(scoresT),
  which also broadcasts the per-q sum across all 128 partitions so the
  normalization multiply needs no cross-partition traffic. No max-subtraction:
  scores are O(5) for RMS-normed q/k, so exp is safe in fp32.
- Causal masking: blocks above the diagonal are skipped outright;
  multiplicative 0/1 masks are applied to the 4 diagonal-band tiles per
  (head, sq-chunk) after exp.
- Matmul operands are bf16 (fp32 PSUM accumulation); the rmsnorm/rope/softmax
  normalization chain stays fp32. Set USE_BF16=False for float32r operands
  (~tf32 precision, 2x slower matmuls).
"""

import numpy as np

import concourse.bass as bass
from concourse import bacc
import concourse.tile as tile
from concourse import mybir
from concourse.bass_utils import run_bass_kernel_spmd
from concourse.masks import make_identity

F32 = mybir.dt.float32
F32R = mybir.dt.float32r
BF16 = mybir.dt.bfloat16

USE_BF16 = True
MM_DT = BF16 if USE_BF16 else F32R
# PE-transpose path dtype: f32r can't be memset/affine_select'd, use plain f32
TR_DT = BF16 if USE_BF16 else F32

S = 2048
HID = 4096
H = 32
KV = 8
D = 128
QH = H // KV  # 4 q heads per kv head; with 8 cores -> 1 kv head per core
NCORES = 8
EPS = 1e-6
THETA = 10000.0
HALF = D // 2

ST = 512  # seq chunk (matmul free dim)
N_ST = S // ST  # 4
KT = HID // D  # 32 contraction tiles for qkv
NB = QH + 2  # 6 feature blocks per core: q0..q3, k, v
SK = S // D  # 16 key blocks of 128
SCALE = float(D) ** -0.5


def build_bass():
    nc = bacc.Bacc(
        "TRN2", target_bir_lowering=False, debug=False, num_devices=NCORES
    )
    hid_d = nc.dram_tensor("hidden_t", [HID, S], MM_DT, kind="ExternalInput").ap()
    wqkv_d = nc.dram_tensor("w_qkvT", [HID, NB * D], MM_DT, kind="ExternalInput").ap()
    wo_d = nc.dram_tensor("w_oT", [QH * D, HID], MM_DT, kind="ExternalInput").ap()
    cosq_d = nc.dram_tensor("cosq", [D, S], F32, kind="ExternalInput").ap()
    sinq_d = nc.dram_tensor("sinq", [D, S], F32, kind="ExternalInput").ap()
    cosk_d = nc.dram_tensor("cosk", [D, S], F32, kind="ExternalInput").ap()
    sink_d = nc.dram_tensor("sink", [D, S], F32, kind="ExternalInput").ap()
    mask_d = nc.dram_tensor("masks", [4, D, ST], MM_DT, kind="ExternalInput").ap()
    out_d = nc.dram_tensor("out_partial", [S, HID], F32, kind="ExternalOutput").ap()

    with tile.TileContext(nc) as tc:
        build_kernel(
            nc, tc, hid_d, wqkv_d, wo_d,
            (cosq_d, sinq_d, cosk_d, sink_d), mask_d, out_d,
        )
    nc.finalize()
    return nc


def build_kernel(nc, tc, hid_d, wqkv_d, wo_d, tables_d, mask_d, out_d):
    from contextlib import ExitStack

    cosq_d, sinq_d, cosk_d, sink_d = tables_d

    with ExitStack() as outer:
        # ---- persistent: q/k in [D, S], v in [Sk, D], constants ----
        qk_pool = outer.enter_context(tc.tile_pool(name="qk", bufs=1))
        const_pool = outer.enter_context(tc.tile_pool(name="const", bufs=1))

        qkT = [qk_pool.tile([D, S], MM_DT, name=f"qkT_{m}") for m in range(QH + 1)]
        v_sb = qk_pool.tile([D, SK, D], MM_DT, name="v_sb")

        identity = const_pool.tile([D, D], TR_DT, name="identity")
        make_identity(nc, identity[:])
        ones_f32 = const_pool.tile([D, D], F32, name="ones_f32")
        nc.vector.memset(ones_f32[:], 1.0)
        ones_sb = const_pool.tile([D, D], MM_DT, name="ones_sb")
        nc.vector.tensor_copy(ones_sb[:], ones_f32[:])
        eps_sb = const_pool.tile([D, 1], F32, name="eps_sb")
        nc.vector.memset(eps_sb[:], EPS)

        # ================= Phase 1: QKV + RMSNorm + RoPE =================
        # Two 3-block waves per seq chunk; each wave's rmsnorm/rope epilogue is
        # emitted AFTER the next wave's matmul batch so the PE never stalls
        # in-order behind the ACT/DVE epilogue chains. PSUM: 3 banks per wave
        # x2 in flight + msum + v-transpose = 8.
        with ExitStack() as p1:
            w_pool = p1.enter_context(tc.tile_pool(name="wqkv", bufs=1))
            hid_pool = p1.enter_context(tc.tile_pool(name="hid", bufs=6))
            tbl_pool = p1.enter_context(tc.tile_pool(name="tbl", bufs=2))
            tmp_pool = p1.enter_context(tc.tile_pool(name="p1tmp", bufs=2))
            ps_pool = p1.enter_context(tc.tile_pool(name="p1ps", bufs=1, space="PSUM"))

            wqkv_sb = w_pool.tile([D, KT, NB * D], MM_DT, name="wqkv_sb")
            for kt in range(KT):
                nc.sync.dma_start(
                    wqkv_sb[:, kt, :], wqkv_d[bass.ts(kt, D), :]
                )

            WAVES = [list(range(0, 3)), list(range(3, NB))]

            def epilogue(st, blocks, qkv_ps, tbls):
                ssl = bass.ts(st, ST)
                cosq_t, sinq_t, cosk_t, sink_t = tbls
                for m in blocks:
                    if m == QH + 1:
                        # v block: transpose [D, ST] -> 4x [128,128] into v_sb
                        vtmp = tmp_pool.tile([D, ST], TR_DT, name="vtmp")
                        nc.vector.tensor_copy(vtmp[:], qkv_ps[m][:])
                        for c in range(ST // D):
                            tp = ps_pool.tile([D, D], TR_DT, name="vt_ps")
                            nc.tensor.transpose(
                                tp[:], vtmp[:, bass.ts(c, D)], identity[:]
                            )
                            nc.vector.tensor_copy(
                                v_sb[:, st * (ST // D) + c, :], tp[:]
                            )
                        continue
                    cos_t, sin_t = (cosq_t, sinq_t) if m < QH else (cosk_t, sink_t)
                    ps = qkv_ps[m]
                    raw = tmp_pool.tile([D, ST], F32, name="raw")
                    nc.scalar.copy(raw[:], ps[:])
                    sq = tmp_pool.tile([D, ST], MM_DT, name="sq")
                    nc.vector.tensor_mul(sq[:], raw[:], raw[:])
                    msum = ps_pool.tile([D, ST], F32, name="msum_ps")
                    nc.tensor.matmul(
                        msum[:], ones_sb[:], sq[:], start=True, stop=True
                    )
                    # rstd = 1/sqrt(mean + eps), already broadcast on partitions
                    rstd = tmp_pool.tile([D, ST], F32, name="rstd")
                    nc.scalar.activation(
                        rstd[:], msum[:], mybir.ActivationFunctionType.Sqrt,
                        bias=eps_sb[:], scale=1.0 / D,
                    )
                    nc.vector.reciprocal_approx_fast(rstd[:], rstd[:])
                    # rope: rot = raw*cos + swap(raw)*sin_signed (gains in
                    # tables; sin table ships half-swapped so both DVE inputs
                    # share a partition base - only the output is relocated)
                    t1 = tmp_pool.tile([D, ST], F32, name="t1")
                    nc.vector.tensor_mul(t1[:], raw[:], cos_t[:])
                    t2 = tmp_pool.tile([D, ST], F32, name="t2")
                    nc.vector.tensor_mul(t2[:HALF], raw[HALF:], sin_t[HALF:])
                    nc.vector.tensor_mul(t2[HALF:], raw[:HALF], sin_t[:HALF])
                    nc.vector.tensor_add(t1[:], t1[:], t2[:])
                    nc.vector.tensor_mul(qkT[m][:, ssl], t1[:], rstd[:])

            pending = None  # (st, blocks, qkv_ps, tbls) awaiting epilogue
            for st in range(N_ST):
                ssl = bass.ts(st, ST)
                tbls = []
                for i, td in enumerate((cosq_d, sinq_d, cosk_d, sink_d)):
                    t = tbl_pool.tile([D, ST], F32, name=f"tbl_{i}")
                    nc.sync.dma_start(t[:], td[:, ssl])
                    tbls.append(t)

                for wave in WAVES:
                    qkv_ps = {
                        m: ps_pool.tile([D, ST], F32, name=f"qkv_ps_{m}")
                        for m in wave
                    }
                    for kt in range(KT):
                        t = hid_pool.tile([D, ST], MM_DT, name="hid_t")
                        nc.sync.dma_start(t[:], hid_d[bass.ts(kt, D), ssl])
                        for m in wave:
                            nc.tensor.matmul(
                                qkv_ps[m][:],
                                wqkv_sb[:, kt, bass.ts(m, D)],
                                t[:],
                                start=(kt == 0),
                                stop=(kt == KT - 1),
                            )
                    if pending is not None:
                        epilogue(*pending)
                    pending = (st, wave, qkv_ps, tbls)
            if pending is not None:
                epilogue(*pending)

        # ========== Phases 2+3 fused: attention + o_proj per chunk ==========
        with ExitStack() as p23:
            ctx_pool = p23.enter_context(tc.tile_pool(name="ctx", bufs=1))
            wo_pool = p23.enter_context(tc.tile_pool(name="wo", bufs=1))
            m_pool = p23.enter_context(tc.tile_pool(name="maskp", bufs=1))
            e_pool = p23.enter_context(tc.tile_pool(name="expp", bufs=6))
            a_tmp = p23.enter_context(tc.tile_pool(name="p2tmp", bufs=3))
            o_pool = p23.enter_context(tc.tile_pool(name="outst", bufs=4))
            sc_ps_pool = p23.enter_context(
                tc.tile_pool(name="sc_ps", bufs=2, space="PSUM")
            )
            cd_ps_pool = p23.enter_context(
                tc.tile_pool(name="cd_ps", bufs=2, space="PSUM")
            )
            o_ps_pool = p23.enter_context(
                tc.tile_pool(name="o_ps", bufs=2, space="PSUM")
            )

            ctx_sb = [ctx_pool.tile([D, S], MM_DT, name=f"ctx_{h}") for h in range(QH)]
            wo_sb = wo_pool.tile([D, QH, HID], MM_DT, name="wo_sb")
            nc.sync.dma_start(wo_sb[:], wo_d.rearrange("(h p) n -> p h n", p=D))
            mask_sb = m_pool.tile([D, 4, ST], MM_DT, name="mask_sb")
            nc.sync.dma_start(mask_sb[:], mask_d.rearrange("r p f -> p r f"))

            def o_proj_chunk(sq):
                # ---- o_proj for the 4 row-tiles of chunk sq ----
                for sti in range(ST // D):
                    st = sq * (ST // D) + sti
                    out_sb = o_pool.tile([D, HID], F32, name="out_sb")
                    for nt in range(HID // ST):
                        ps = o_ps_pool.tile([D, ST], F32, name="o_ps")
                        for ht in range(QH):
                            nc.tensor.matmul(
                                ps[:],
                                ctx_sb[ht][:, bass.ts(st, D)],
                                wo_sb[:, ht, bass.ts(nt, ST)],
                                start=(ht == 0),
                                stop=(ht == QH - 1),
                            )
                        nc.vector.tensor_copy(out_sb[:, bass.ts(nt, ST)], ps[:])
                    nc.sync.dma_start(out_d[bass.ts(st, D), :], out_sb[:])

            for sq in range(N_ST):
                ssl = bass.ts(sq, ST)
                n_sk = (sq + 1) * (ST // D)
                # ---- attention for all heads on this chunk ----
                for h in range(QH):
                    ctx_ps = cd_ps_pool.tile([D, ST], F32, name="ctx_ps")
                    den_ps = cd_ps_pool.tile([D, ST], F32, name="den_ps")
                    for ski in range(n_sk):
                        sc = sc_ps_pool.tile([D, ST], F32, name="sc_ps")
                        nc.tensor.matmul(
                            sc[:],
                            qkT[QH][:, bass.ts(ski, D)],
                            qkT[h][:, ssl],
                            start=True,
                            stop=True,
                        )
                        e_sb = e_pool.tile([D, ST], MM_DT, name="e_sb")
                        nc.scalar.activation(
                            e_sb[:], sc[:],
                            mybir.ActivationFunctionType.Exp,
                            scale=SCALE,
                        )
                        r = ski - sq * (ST // D)
                        if r >= 0:
                            nc.vector.tensor_mul(
                                e_sb[:], e_sb[:], mask_sb[:, r, :]
                            )
                        first = ski == 0
                        last = ski == n_sk - 1
                        nc.tensor.matmul(
                            ctx_ps[:], v_sb[:, ski, :], e_sb[:],
                            start=first, stop=last,
                        )
                        nc.tensor.matmul(
                            den_ps[:], ones_sb[:], e_sb[:],
                            start=first, stop=last,
                        )
                    recip = a_tmp.tile([D, ST], F32, name="recip")
                    nc.vector.reciprocal_approx_fast(recip[:], den_ps[:])
                    nc.vector.tensor_mul(ctx_sb[h][:, ssl], ctx_ps[:], recip[:])

                # o_proj lags attention by one chunk so the normalization
                # chain of this chunk hides under the next chunk's matmuls
                if sq > 0:
                    o_proj_chunk(sq - 1)
            o_proj_chunk(N_ST - 1)


def _host_prep(positions, hidden_states, w_qkv, w_o, gq, gk):
    import ml_dtypes

    mm_np = ml_dtypes.bfloat16 if USE_BF16 else np.float32

    positions = np.asarray(positions)
    hidden_states = np.asarray(hidden_states, dtype=np.float32)
    w_qkv = np.asarray(w_qkv, dtype=np.float32)
    w_o = np.asarray(w_o, dtype=np.float32)
    gq = np.asarray(gq, dtype=np.float32)
    gk = np.asarray(gk, dtype=np.float32)

    hidden_t = np.ascontiguousarray(hidden_states.T).astype(mm_np)  # [HID, S]

    inv_freq = 1.0 / (THETA ** (np.arange(HALF, dtype=np.float32) * 2.0 / D))
    freqs = positions.astype(np.float32)[:, None] * inv_freq  # [S, HALF]
    cos = np.cos(freqs)
    sin = np.sin(freqs)

    def rope_tables(g):
        # cos_t[d, s] = cos[s, d % HALF] * g[d]
        cos_t = np.concatenate([cos * g[None, :HALF], cos * g[None, HALF:]], axis=1).T
        # rot[d] = x[d] cos[d] - x[d+HALF] sin[d]   (d < HALF)
        #          x[d] cos[d] + x[d-HALF] sin[d]   (d >= HALF)
        # The partner's gain is folded into the sin table, and the table ships
        # HALF-SWAPPED: row j holds the sin factor that multiplies x[j], i.e.
        # the factor for output row j^HALF. This keeps both DVE inputs of the
        # rope cross-multiply at the same partition base.
        sin_t = np.concatenate([sin * g[None, :HALF], -sin * g[None, HALF:]], axis=1).T
        return (
            np.ascontiguousarray(cos_t, dtype=np.float32),
            np.ascontiguousarray(sin_t, dtype=np.float32),
        )

    cosq, sinq = rope_tables(gq)
    cosk, sink = rope_tables(gk)

    # diagonal-band masks: mask[r][k, q] = 1 if (r*D + k) <= q else 0
    k_idx = np.arange(D)
    q_idx = np.arange(ST)
    masks = np.stack(
        [
            ((r * D + k_idx)[:, None] <= q_idx[None, :]).astype(mm_np)
            for r in range(4)
        ]
    )

    per_core = []
    for c in range(NCORES):
        q_rows = w_qkv[c * QH * D : (c + 1) * QH * D]  # [512, HID]
        k_rows = w_qkv[H * D + c * D : H * D + (c + 1) * D]  # [128, HID]
        v_rows = w_qkv[(H + KV) * D + c * D : (H + KV) * D + (c + 1) * D]
        wqkv_c = np.concatenate([q_rows, k_rows, v_rows], axis=0)  # [768, HID]
        wqkv_t = np.ascontiguousarray(wqkv_c.T).astype(mm_np)  # [HID, 768]
        wo_t = np.ascontiguousarray(
            w_o[:, c * QH * D : (c + 1) * QH * D].T
        ).astype(mm_np)
        per_core.append(
            {
                "hidden_t": hidden_t,
                "w_qkvT": wqkv_t,
                "w_oT": wo_t,
                "cosq": cosq,
                "sinq": sinq,
                "cosk": cosk,
                "sink": sink,
                "masks": masks,
            }
        )
    return per_core


_NC_CACHE = {}


def _get_nc():
    if "nc" not in _NC_CACHE:
        _NC_CACHE["nc"] = build_bass()
    return _NC_CACHE["nc"]


def kernel(positions, hidden_states, w_qkv, w_o, gq, gk, _trace=False):
    in_maps = _host_prep(positions, hidden_states, w_qkv, w_o, gq, gk)
    nc = _get_nc()
    res = run_bass_kernel_spmd(
        nc, in_maps, core_ids=list(range(NCORES)), trace=_trace
    )
    out = np.zeros((S, HID), dtype=np.float32)
    for r in res.results:
        out += r["out_partial"]
    if _trace:
        kernel._last_results = res
    return out
